# revision 2
# baseline (speedup 1.0000x reference)
"""Trainium2 Bass kernel: GNN message passing  out = relu((adj @ x) @ W.T + b).

Reassociated as  out = relu(adj @ y + b)  with y = x @ W.T folded into host
prep (0.8% of the FLOPs), so the device runs a single big matmul.  That
matmul runs in fp8e4 DoubleRow mode (2 weights per PE cell, 256-deep
contraction per instruction, ~216 ns per [256x128]x[256x512] MM = fp8 peak).
fp8 error is held at ~1.8e-2 by two exact algebraic corrections folded into
the bias:
  * adj is mean-centered (B = adj - 0.5), halving its quantization noise;
    the rank-1 term 0.5 * colsum(y) is exact.
  * using colsum(y_true) rather than colsum(y_fp8) also cancels the
    coherent (mean) component of y's quantization error, halving it.

Sharding: 1D row partition of adj across 8 NeuronCores (1024 rows each);
each core computes outT = y.T @ adjT_c with y-tiles stationary and the
centered adj shard streaming, accumulating f32 in all 8 PSUM banks over
the 8192-deep contraction, then fuses bias+ReLU on PSUM eviction.

v2 structure (from NTFF profile analysis of the 77.9 us baseline):
the PE body was already 97% dense at the 216 ns/MM fp8 roofline, so the
remaining headroom is the head (DMA ring ramp + cold-clock MMs) and the
tail (serialized evictions + eight 128 KB stores):
  * adj is one flat per-partition-contiguous DRAM region; the whole 8 MiB
    shard is loaded into SBUF with 10 large HWDGE DMAs (0.25-1 MiB) on the
    sync queue -- large transfers run ~341 GB/s vs ~175 GB/s for the
    baseline's 64x128 KB streams, so the stream stays far ahead of the PE.
  * y (4 MiB) loads in 4 large chunks on the scalar queue.
  * the MM stream is split into two phases by output-row half (mc): phase
    A's four PSUM banks evict + store (512 KB) in the middle of the
    kernel, hidden under phase B's MMs; only phase B's 512 KB remains in
    the tail, stored as two parallel 256 KB DMAs on the two HWDGE queues.
  * dummy matmuls on a small zeroed scratch keep the PE busy through the
    HAM ramp so the clock gate reaches 8/8 before real data lands; they
    open bank (0,0)'s accumulation group and add exact zeros, so no group
    boundary sits between them and the real stream.
Output is stored bf16 to halve the post-matmul store drain.
"""

import numpy as np
import ml_dtypes

import concourse.mybir as mybir
from concourse import bacc
from concourse.tile import TileContext
from concourse.bass_utils import run_bass_kernel_spmd

P = 128
N_NODES = 8192
DIM = 512
NCORES = 8
M = N_NODES // NCORES          # 1024 output rows per core
KT = N_NODES // P              # 64 contraction tiles of 128
DT = KT // 2                   # 32 DoubleRow tiles (256-deep each)
NT = DIM // P                  # 4 tiles of the feature dim (output part.)
FREE = 512                     # moving free dim / PSUM bank width (f32)
MCH = M // FREE                # 2 moving chunks (phases) per core
TB = MCH * 2 * FREE            # adj bytes per partition per t-tile (2048)
WARM = 12                      # HAM warm-up matmuls on scratch
FP8 = mybir.dt.float8e4
F32 = mybir.dt.float32
BF16 = mybir.dt.bfloat16
DR = mybir.MatmulPerfMode.DoubleRow

# adj chunk boundaries in t units: small first chunks so the PE starts
# early, 1 MiB steady-state chunks for DMA efficiency.
ADJ_CHUNKS = [(0, 1), (1, 2), (2, 4), (4, 8), (8, 12), (12, 16),
              (16, 20), (20, 24), (24, 28), (28, 32)]
# y chunk boundaries in k units (k-tile of 128 rows); MM tile t needs
# k = 2t, 2t+1.
Y_CHUNKS = [(0, 4), (4, 16), (16, 40), (40, 64)]

_NC = None


def _build_nc():
    nc = bacc.Bacc("TRN2", debug=False)
    # yp[p, k*DIM+d] = y[k*128+p, d]
    yp_d = nc.dram_tensor("yp", [P, KT * DIM], FP8, kind="ExternalInput").ap()
    # adjp[p, t*2048 + mc*1024 + i*512 + m] = B.T[(2t+i)*128+p, mc*512+m]
    adjp_d = nc.dram_tensor("adjp", [P, DT * TB], FP8, kind="ExternalInput").ap()
    cb_d = nc.dram_tensor("cb", [P, NT], F32, kind="ExternalInput").ap()
    # out4[mc, h, p, nl*512+r] = outT[(2h+nl)*128+p, mc*512+r]
    out_d = nc.dram_tensor(
        "out4", [MCH, 2, P, 2 * FREE], BF16, kind="ExternalOutput"
    ).ap()

    with TileContext(nc) as tc:
        with (
            tc.tile_pool(name="sb", bufs=1) as pool,
            tc.tile_pool(name="ps", bufs=8, space="PSUM") as pspool,
        ):
            adj_sb = {}
            for ci, (t0, t1) in enumerate(ADJ_CHUNKS):
                adj_sb[ci] = pool.tile(
                    [P, t1 - t0, MCH, 2, FREE], FP8, tag=f"adj{ci}", name=f"adj{ci}"
                )
            y_sb = {}
            for ci, (k0, k1) in enumerate(Y_CHUNKS):
                y_sb[ci] = pool.tile(
                    [P, k1 - k0, DIM], FP8, tag=f"y{ci}", name=f"y{ci}"
                )
            cb_sb = pool.tile([P, NT], F32, tag="cb")
            scr_sb = pool.tile([P, 2, 2 * P], FP8, tag="scr")
            # bf16 store staging: two half-tiles per phase
            o_sb = [
                [
                    pool.tile([P, 2, FREE], BF16, tag=f"o{mc}{h}", name=f"o{mc}{h}")
                    for h in range(2)
                ]
                for mc in range(MCH)
            ]

            agg_ps = [
                [
                    pspool.tile([P, FREE], F32, tag="ps", name=f"ps_{mc}_{n}")
                    for n in range(NT)
                ]
                for mc in range(MCH)
            ]

            # Input streams: adj on sync (HWDGE), y on scalar (HWDGE),
            # bias on gpsimd.  Program order = queue FIFO order.
            for ci, (t0, t1) in enumerate(ADJ_CHUNKS):
                nc.sync.dma_start(adj_sb[ci][:], adjp_d[:, t0 * TB : t1 * TB])
            for ci, (k0, k1) in enumerate(Y_CHUNKS):
                nc.scalar.dma_start(y_sb[ci][:], yp_d[:, k0 * DIM : k1 * DIM])
            nc.gpsimd.dma_start(cb_sb[:], cb_d[:])

            nc.vector.memset(scr_sb[:], 0)

            # Dummy matmuls on zeroed scratch keep the PE busy from the
            # end of its preamble so the HAM clock gate reaches 8/8 before
            # real data lands.  They open bank (0,0)'s accumulation group
            # and add exact zeros, so no group boundary (and no scheduler
            # wait) sits between them and the real stream.
            for w in range(WARM):
                nc.tensor.matmul(
                    agg_ps[0][0][:, 0:P],
                    scr_sb[:, :, :P],
                    scr_sb[:, :, P:],
                    start=(w == 0),
                    stop=False,
                    perf_mode=DR,
                )

            def t_chunk(t):
                for ci, (t0, t1) in enumerate(ADJ_CHUNKS):
                    if t0 <= t < t1:
                        return ci, t0
                raise AssertionError

            def k_chunk(k):
                for ci, (k0, k1) in enumerate(Y_CHUNKS):
                    if k0 <= k < k1:
                        return ci, k0
                raise AssertionError

            for mc in range(MCH):
                for t in range(DT):
                    aci, at0 = t_chunk(t)
                    yci, yk0 = k_chunk(2 * t)
                    for n in range(NT):
                        nc.tensor.matmul(
                            agg_ps[mc][n][:],
                            y_sb[yci][:, 2 * t - yk0 : 2 * t - yk0 + 2,
                                       n * P : (n + 1) * P],
                            adj_sb[aci][:, t - at0, mc],
                            start=(t == 0) and not (mc == 0 and n == 0),
                            stop=(t == DT - 1),
                            perf_mode=DR,
                        )
                # Phase epilogue: bias+ReLU on PSUM eviction, ACT/DVE
                # alternating; each staged half stored as one 256 KB DMA.
                # Phase A (mc=0) stores sit under phase B's MMs on the
                # gpsimd queue; phase B's two stores go on the two HWDGE
                # queues (long idle by then) to drain in parallel.
                for n in range(NT):
                    dst = o_sb[mc][n // 2][:, n % 2]
                    if n % 2 == 0:
                        nc.scalar.activation(
                            dst,
                            agg_ps[mc][n][:],
                            mybir.ActivationFunctionType.Relu,
                            bias=cb_sb[:, n : n + 1],
                        )
                    else:
                        nc.vector.tensor_scalar(
                            dst,
                            agg_ps[mc][n][:],
                            cb_sb[:, n : n + 1],
                            0.0,
                            mybir.AluOpType.add,
                            mybir.AluOpType.max,
                        )
                if mc == 0:
                    nc.gpsimd.dma_start(out_d[0, 0], o_sb[0][0][:])
                    nc.gpsimd.dma_start(out_d[0, 1], o_sb[0][1][:])
                else:
                    nc.sync.dma_start(out_d[1, 0], o_sb[1][0][:])
                    nc.scalar.dma_start(out_d[1, 1], o_sb[1][1][:])
    nc.finalize()
    return nc


def _get_nc():
    global _NC
    if _NC is None:
        _NC = _build_nc()
    return _NC


def _prepare(inputs):
    e4 = ml_dtypes.float8_e4m3
    x = np.asarray(inputs["x"], dtype=np.float32)
    adj = np.asarray(inputs["adj"], dtype=np.float32)
    W = np.asarray(inputs["W"], dtype=np.float32)
    b = np.asarray(inputs["b"], dtype=np.float64)

    y = x @ W.T.astype(np.float32)
    y8 = y.astype(e4)
    # bias fold: nn bias + exact centering/rank-1 correction term
    c = (b + 0.5 * y.astype(np.float64).sum(axis=0)).astype(np.float32)
    cb_tiled = np.ascontiguousarray(c.reshape(NT, P).T)  # [128, 4]

    # y pre-tiled so every y DMA is flat: yp[p, k*DIM+d] = y8[k*128+p, d]
    yp = np.ascontiguousarray(
        y8.reshape(KT, P, DIM).transpose(1, 0, 2).reshape(P, KT * DIM)
    )

    B8T = (adj - np.float32(0.5)).astype(e4).T  # [K, rows] view

    in_maps = []
    for ci in range(NCORES):
        # adjp[p, t*2048 + mc*1024 + i*512 + m] = B8T[(2t+i)*128+p, ci*M+mc*512+m]
        shard = np.ascontiguousarray(B8T[:, ci * M : (ci + 1) * M])
        adjp = np.ascontiguousarray(
            shard.reshape(DT, 2, P, MCH, FREE).transpose(2, 0, 3, 1, 4)
        ).reshape(P, DT * TB)
        in_maps.append({"yp": yp, "adjp": adjp, "cb": cb_tiled})
    return in_maps


def _run(in_maps, **kwargs):
    return run_bass_kernel_spmd(
        _get_nc(), in_maps, core_ids=list(range(NCORES)), **kwargs
    )


def _assemble(results):
    out = np.empty((N_NODES, DIM), dtype=np.float32)
    for ci in range(NCORES):
        o4 = results[ci]["out4"].astype(np.float32)  # [MCH, 2, P, 1024]
        # element [mc, h, p, nl*512+r] = out[ci*1024 + mc*512 + r, (2h+nl)*128+p]
        blk = (
            o4.reshape(MCH, 2, P, 2, FREE)
            .transpose(0, 4, 1, 3, 2)
            .reshape(M, DIM)
        )
        out[ci * M : (ci + 1) * M, :] = blk
    return out


def kernel(**inputs):
    res = _run(_prepare(inputs))
    return _assemble(res.results)


# revision 9
# speedup vs baseline: 1.0302x; 1.0302x over previous
"""Trainium2 Bass kernel: GNN message passing  out = relu((adj @ x) @ W.T + b).

Reassociated as  out = relu(adj @ y + b)  with y = x @ W.T folded into host
prep (0.8% of the FLOPs), so the device runs a single big matmul.  That
matmul runs in fp8e4 DoubleRow mode (2 weights per PE cell, 256-deep
contraction per instruction, ~216 ns per [256x128]x[256x512] MM = fp8 peak).
fp8 error is held at ~1.8e-2 by two exact algebraic corrections folded into
the bias:
  * adj is mean-centered (B = adj - 0.5), halving its quantization noise;
    the rank-1 term 0.5 * colsum(y) is exact.
  * using colsum(y_true) rather than colsum(y_fp8) also cancels the
    coherent (mean) component of y's quantization error, halving it.

Sharding: 1D row partition of adj across 8 NeuronCores (1024 rows each);
each core computes outT = y.T @ adjT_c with y-tiles stationary and the
centered adj shard streaming, accumulating f32 in all 8 PSUM banks over
the 8192-deep contraction, then fuses bias+ReLU on PSUM eviction.

v3 structure (from NTFF profile analysis; baseline 77.9 us had a 97%-dense
PE body already at the fp8 roofline, so the wins are head + tail + DMA
efficiency):
  * adj is one flat per-partition-contiguous DRAM region streamed with a
    few 0.25-1 MiB HWDGE DMAs on the sync queue (~300+ GB/s vs ~175 GB/s
    for the baseline's 64x128 KB streams).  The whole 8 MiB shard is
    resident in SBUF; single pass over t keeps demand at ~220 GB/s.
  * the DMA ring does not move bytes before ~8.7 us regardless of issue
    time, so the first transfers are exactly the first tile's needs
    (y[k0:2], adj[t0,mc0], adj[t0,mc1] = 3x128 KB) and the first real MM
    fires ~10.6 us; 18 warm-up matmuls on zeroed scratch keep the PE busy
    from ~8.0 us so the HAM clock gate is at 8/8 when real data lands.
  * ALL input chunks ride the single sync HWDGE queue interleaved in
    consumption order (y[2t0:2t1] before adj[t0:t1]): the SDMA packet
    round-robin between queues is grossly unfair (big-transfer queues win
    ~10:1), so a second queue for y starves the adj chunks the PE needs
    next, while one queue alone still drains at full rate.
  * tail: the last t-tile's 8 MMs stop banks in (n, mc) order; evictions
    pair same-n banks (DVE takes mc=0 which stops first, ACT mc=1) into
    four [128,2,512] bf16 staging tiles, each stored as its own 256 KB
    DMA on a separate queue (gpsimd/vector/sync/scalar) as soon as its
    pair completes, overlapping the drains.
Output is stored bf16 to halve the post-matmul store drain.
"""

import numpy as np
import ml_dtypes

import concourse.mybir as mybir
from concourse import bacc
from concourse.tile import TileContext
from concourse.bass_utils import run_bass_kernel_spmd

P = 128
N_NODES = 8192
DIM = 512
NCORES = 8
M = N_NODES // NCORES          # 1024 output rows per core
KT = N_NODES // P              # 64 contraction tiles of 128
DT = KT // 2                   # 32 DoubleRow tiles (256-deep each)
NT = DIM // P                  # 4 tiles of the feature dim (output part.)
FREE = 512                     # moving free dim / PSUM bank width (f32)
MCH = M // FREE                # 2 moving chunks per t-tile
TB = MCH * 2 * FREE            # adj bytes per partition per t-tile (2048)
WARM = 18                      # HAM warm-up matmuls on scratch
FP8 = mybir.dt.float8e4
F32 = mybir.dt.float32
BF16 = mybir.dt.bfloat16
DR = mybir.MatmulPerfMode.DoubleRow

# adj chunk boundaries in t units (t=0 is loaded separately as two 128 KB
# mc-halves): small early chunks so the PE starts early, ~1 MiB
# steady-state chunks for DMA efficiency.
ADJ_CHUNKS = [(1, 2), (2, 4), (4, 8), (8, 12), (12, 16),
              (16, 20), (20, 24), (24, 28), (28, 32)]
# y chunk boundaries in k units (k-tile of 128 rows); MM tile t needs
# k = 2t, 2t+1.  Chunk c covers exactly adj chunk c-1's k-range so the
# single-queue FIFO delivers in consumption order.
Y_CHUNKS = [(0, 2)] + [(2 * t0, 2 * t1) for t0, t1 in ADJ_CHUNKS]

_NC = None


def _build_nc():
    nc = bacc.Bacc("TRN2", debug=False)
    # yp[p, k*DIM+d] = y[k*128+p, d]
    yp_d = nc.dram_tensor("yp", [P, KT * DIM], FP8, kind="ExternalInput").ap()
    # adjp[p, t*2048 + mc*1024 + i*512 + m] = B.T[(2t+i)*128+p, mc*512+m]
    adjp_d = nc.dram_tensor("adjp", [P, DT * TB], FP8, kind="ExternalInput").ap()
    cb_d = nc.dram_tensor("cb", [P, NT], F32, kind="ExternalInput").ap()
    # out5[n, p, mc*512+r] = outT[n*128+p, mc*512+r]  (partition dim first
    # within each n-block so the store DMA is 128 x 2 KB clean descriptors)
    out_d = nc.dram_tensor(
        "out5", [NT, P, MCH * FREE], BF16, kind="ExternalOutput"
    ).ap()

    with TileContext(nc) as tc:
        with (
            tc.tile_pool(name="sb", bufs=1) as pool,
            tc.tile_pool(name="ps", bufs=8, space="PSUM") as pspool,
        ):
            # t=0 arrives as two 128 KB mc-halves so the first MM is gated
            # on the minimum possible bytes.
            adj0 = [
                pool.tile([P, 2, FREE], FP8, tag=f"adj0{mc}", name=f"adj0{mc}")
                for mc in range(MCH)
            ]
            adj_sb = {}
            for ci, (t0, t1) in enumerate(ADJ_CHUNKS):
                adj_sb[ci] = pool.tile(
                    [P, t1 - t0, MCH, 2, FREE], FP8, tag=f"adj{ci}", name=f"adj{ci}"
                )
            y_sb = {}
            for ci, (k0, k1) in enumerate(Y_CHUNKS):
                y_sb[ci] = pool.tile(
                    [P, k1 - k0, DIM], FP8, tag=f"y{ci}", name=f"y{ci}"
                )
            cb_sb = pool.tile([P, NT], F32, tag="cb")
            scr_sb = pool.tile([P, 2, 2 * P], FP8, tag="scr")
            # bf16 store staging: one [128, 2, 512] tile per feature block
            o_sb = [
                pool.tile([P, MCH, FREE], BF16, tag=f"o{n}", name=f"o{n}")
                for n in range(NT)
            ]

            agg_ps = [
                [
                    pspool.tile([P, FREE], F32, tag="ps", name=f"ps_{mc}_{n}")
                    for n in range(NT)
                ]
                for mc in range(MCH)
            ]

            # The whole input stream rides the ONE sync HWDGE queue in
            # exact consumption order: the SDMA packet round-robin between
            # queues is grossly unfair (big-transfer queues win ~10:1), so
            # a second queue for y starves the adj chunks the PE needs
            # next.  Single-queue FIFO makes arrival order deterministic
            # and one queue alone drains at full rate (~390 GB/s warm).
            nc.sync.dma_start(y_sb[0][:], yp_d[:, 0 : 2 * DIM])
            nc.sync.dma_start(adj0[0][:], adjp_d[:, 0 : TB // 2])
            nc.sync.dma_start(adj0[1][:], adjp_d[:, TB // 2 : TB])
            for ci, (t0, t1) in enumerate(ADJ_CHUNKS):
                k0, k1 = Y_CHUNKS[ci + 1]
                nc.sync.dma_start(y_sb[ci + 1][:], yp_d[:, k0 * DIM : k1 * DIM])
                nc.sync.dma_start(adj_sb[ci][:], adjp_d[:, t0 * TB : t1 * TB])
            nc.gpsimd.dma_start(cb_sb[:], cb_d[:])

            nc.vector.memset(scr_sb[:], 0)

            # Dummy matmuls on zeroed scratch keep the PE busy from the
            # end of its preamble so the HAM clock gate reaches 8/8 before
            # real data lands.  They open bank (0,0)'s accumulation group
            # and add exact zeros, so no group boundary (and no scheduler
            # wait) sits between them and the real stream.
            for w in range(WARM):
                nc.tensor.matmul(
                    agg_ps[0][0][:, 0:P],
                    scr_sb[:, :, :P],
                    scr_sb[:, :, P:],
                    start=(w == 0),
                    stop=False,
                    perf_mode=DR,
                )

            def t_chunk(t):
                for ci, (t0, t1) in enumerate(ADJ_CHUNKS):
                    if t0 <= t < t1:
                        return ci, t0
                raise AssertionError

            def k_chunk(k):
                for ci, (k0, k1) in enumerate(Y_CHUNKS):
                    if k0 <= k < k1:
                        return ci, k0
                raise AssertionError

            for t in range(DT):
                yci, yk0 = k_chunk(2 * t)
                for n in range(NT):
                    y_ap = y_sb[yci][:, 2 * t - yk0 : 2 * t - yk0 + 2,
                                     n * P : (n + 1) * P]
                    for mc in range(MCH):
                        if t == 0:
                            a_ap = adj0[mc][:]
                        else:
                            aci, at0 = t_chunk(t)
                            a_ap = adj_sb[aci][:, t - at0, mc]
                        nc.tensor.matmul(
                            agg_ps[mc][n][:],
                            y_ap,
                            a_ap,
                            start=(t == 0) and not (n == 0 and mc == 0),
                            stop=(t == DT - 1),
                            perf_mode=DR,
                        )

            # Epilogue: bias+ReLU on PSUM eviction.  Bank (mc, n) stops at
            # the (n, mc)-ordered last t-tile MMs; DVE takes mc=0 (stops
            # first, slower op), ACT mc=1.  Each same-n pair stores as one
            # 256 KB DMA on its own queue the moment both halves land.
            store_eng = [nc.gpsimd, nc.gpsimd, nc.sync, nc.scalar]
            for n in range(NT):
                for mc in range(MCH):
                    dst = o_sb[n][:, mc]
                    if mc == 0:
                        nc.vector.tensor_scalar(
                            dst,
                            agg_ps[mc][n][:],
                            cb_sb[:, n : n + 1],
                            0.0,
                            mybir.AluOpType.add,
                            mybir.AluOpType.max,
                        )
                    else:
                        nc.scalar.activation(
                            dst,
                            agg_ps[mc][n][:],
                            mybir.ActivationFunctionType.Relu,
                            bias=cb_sb[:, n : n + 1],
                        )
                store_eng[n].dma_start(out_d[n], o_sb[n][:])
    nc.finalize()
    return nc


def _get_nc():
    global _NC
    if _NC is None:
        _NC = _build_nc()
    return _NC


def _prepare(inputs):
    e4 = ml_dtypes.float8_e4m3
    x = np.asarray(inputs["x"], dtype=np.float32)
    adj = np.asarray(inputs["adj"], dtype=np.float32)
    W = np.asarray(inputs["W"], dtype=np.float32)
    b = np.asarray(inputs["b"], dtype=np.float64)

    y = x @ W.T.astype(np.float32)
    y8 = y.astype(e4)
    # bias fold: nn bias + exact centering/rank-1 correction term
    c = (b + 0.5 * y.astype(np.float64).sum(axis=0)).astype(np.float32)
    cb_tiled = np.ascontiguousarray(c.reshape(NT, P).T)  # [128, 4]

    # y pre-tiled so every y DMA is flat: yp[p, k*DIM+d] = y8[k*128+p, d]
    yp = np.ascontiguousarray(
        y8.reshape(KT, P, DIM).transpose(1, 0, 2).reshape(P, KT * DIM)
    )

    B8T = (adj - np.float32(0.5)).astype(e4).T  # [K, rows] view

    in_maps = []
    for ci in range(NCORES):
        # adjp[p, t*2048 + mc*1024 + i*512 + m] = B8T[(2t+i)*128+p, ci*M+mc*512+m]
        shard = np.ascontiguousarray(B8T[:, ci * M : (ci + 1) * M])
        adjp = np.ascontiguousarray(
            shard.reshape(DT, 2, P, MCH, FREE).transpose(2, 0, 3, 1, 4)
        ).reshape(P, DT * TB)
        in_maps.append({"yp": yp, "adjp": adjp, "cb": cb_tiled})
    return in_maps


def _run(in_maps, **kwargs):
    return run_bass_kernel_spmd(
        _get_nc(), in_maps, core_ids=list(range(NCORES)), **kwargs
    )


def _assemble(results):
    out = np.empty((N_NODES, DIM), dtype=np.float32)
    for ci in range(NCORES):
        o5 = results[ci]["out5"].astype(np.float32)  # [NT, P, MCH*FREE]
        # element [n, p, j] = out[ci*1024 + j, n*128 + p]
        blk = o5.transpose(2, 0, 1).reshape(M, DIM)
        out[ci * M : (ci + 1) * M, :] = blk
    return out


def kernel(**inputs):
    res = _run(_prepare(inputs))
    return _assemble(res.results)


# revision 12
# speedup vs baseline: 1.0524x; 1.0215x over previous
"""Trainium2 Bass kernel: GNN message passing  out = relu((adj @ x) @ W.T + b).

Reassociated as  out = relu(adj @ y + b)  with y = x @ W.T folded into host
prep (0.8% of the FLOPs), so the device runs a single big matmul.  That
matmul runs in fp8e4 DoubleRow mode (2 weights per PE cell, 256-deep
contraction per instruction, ~216 ns per [256x128]x[256x512] MM = fp8 peak).
fp8 error is held at ~1.8e-2 by two exact algebraic corrections folded into
the bias:
  * adj is mean-centered (B = adj - 0.5), halving its quantization noise;
    the rank-1 term 0.5 * colsum(y) is exact.
  * using colsum(y_true) rather than colsum(y_fp8) also cancels the
    coherent (mean) component of y's quantization error, halving it.

Sharding: 1D row partition of adj across 8 NeuronCores (1024 rows each);
each core computes outT = y.T @ adjT_c with y-tiles stationary and the
centered adj shard streaming, accumulating f32 in all 8 PSUM banks over
the 8192-deep contraction, then fuses bias+ReLU on PSUM eviction.

v3 structure (from NTFF profile analysis; baseline 77.9 us had a 97%-dense
PE body already at the fp8 roofline, so the wins are head + tail + DMA
efficiency):
  * adj is one flat per-partition-contiguous DRAM region streamed with a
    few 0.25-1 MiB HWDGE DMAs on the sync queue (~300+ GB/s vs ~175 GB/s
    for the baseline's 64x128 KB streams).  The whole 8 MiB shard is
    resident in SBUF; single pass over t keeps demand at ~220 GB/s.
  * the DMA ring does not move bytes before ~8.7 us regardless of issue
    time, so the first transfers are exactly the first tile's needs
    (y[k0:2], adj[t0,mc0], adj[t0,mc1] = 3x128 KB) and the first real MM
    fires ~10.6 us; 18 warm-up matmuls on zeroed scratch keep the PE busy
    from ~8.0 us so the HAM clock gate is at 8/8 when real data lands.
  * ALL input chunks ride the single sync HWDGE queue interleaved in
    consumption order (y[2t0:2t1] before adj[t0:t1]): the SDMA packet
    round-robin between queues is grossly unfair (big-transfer queues win
    ~10:1), so a second queue for y starves the adj chunks the PE needs
    next, while one queue alone still drains at full rate.
  * tail: the last t-tile's 8 MMs stop banks in (n, mc) order; evictions
    pair same-n banks (DVE takes mc=0 which stops first, ACT mc=1) into
    four [128,2,512] bf16 staging tiles, each stored as its own 256 KB
    DMA on a separate queue (gpsimd/vector/sync/scalar) as soon as its
    pair completes, overlapping the drains.
Output is stored bf16 to halve the post-matmul store drain.
"""

import numpy as np
import ml_dtypes

import concourse.mybir as mybir
from concourse import bacc
from concourse.tile import TileContext
from concourse.bass_utils import run_bass_kernel_spmd

P = 128
N_NODES = 8192
DIM = 512
NCORES = 8
M = N_NODES // NCORES          # 1024 output rows per core
KT = N_NODES // P              # 64 contraction tiles of 128
DT = KT // 2                   # 32 DoubleRow tiles (256-deep each)
NT = DIM // P                  # 4 tiles of the feature dim (output part.)
FREE = 512                     # moving free dim / PSUM bank width (f32)
MCH = M // FREE                # 2 moving chunks per t-tile
TB = MCH * 2 * FREE            # adj bytes per partition per t-tile (2048)
WARM = 18                      # HAM warm-up matmuls on scratch
FP8 = mybir.dt.float8e4
F32 = mybir.dt.float32
BF16 = mybir.dt.bfloat16
DR = mybir.MatmulPerfMode.DoubleRow

# adj chunk boundaries in t units (t=0 is loaded separately as two 128 KB
# mc-halves): small early chunks so the PE starts early, ~1 MiB
# steady-state chunks for DMA efficiency.
ADJ_CHUNKS = [(1, 2), (2, 3), (3, 4), (4, 6), (6, 8), (8, 12), (12, 16),
              (16, 20), (20, 24), (24, 28), (28, 32)]
# y chunk boundaries in k units (k-tile of 128 rows); MM tile t needs
# k = 2t, 2t+1.  Chunk c covers exactly adj chunk c-1's k-range so the
# single-queue FIFO delivers in consumption order.
Y_CHUNKS = [(0, 2)] + [(2 * t0, 2 * t1) for t0, t1 in ADJ_CHUNKS]

_NC = None


def _build_nc():
    nc = bacc.Bacc("TRN2", debug=False)
    # yp[p, k*DIM+d] = y[k*128+p, d]
    yp_d = nc.dram_tensor("yp", [P, KT * DIM], FP8, kind="ExternalInput").ap()
    # adjp[p, t*2048 + mc*1024 + i*512 + m] = B.T[(2t+i)*128+p, mc*512+m]
    adjp_d = nc.dram_tensor("adjp", [P, DT * TB], FP8, kind="ExternalInput").ap()
    cb_d = nc.dram_tensor("cb", [P, NT], F32, kind="ExternalInput").ap()
    # out5[n, p, mc*512+r] = outT[n*128+p, mc*512+r]  (partition dim first
    # within each n-block so the store DMA is 128 x 2 KB clean descriptors)
    out_d = nc.dram_tensor(
        "out5", [NT, P, MCH * FREE], BF16, kind="ExternalOutput"
    ).ap()

    with TileContext(nc) as tc:
        with (
            tc.tile_pool(name="sb", bufs=1) as pool,
            tc.tile_pool(name="ps", bufs=8, space="PSUM") as pspool,
        ):
            # t=0 arrives as two 128 KB mc-halves so the first MM is gated
            # on the minimum possible bytes.
            adj0 = [
                pool.tile([P, 2, FREE], FP8, tag=f"adj0{mc}", name=f"adj0{mc}")
                for mc in range(MCH)
            ]
            adj_sb = {}
            for ci, (t0, t1) in enumerate(ADJ_CHUNKS):
                adj_sb[ci] = pool.tile(
                    [P, t1 - t0, MCH, 2, FREE], FP8, tag=f"adj{ci}", name=f"adj{ci}"
                )
            y_sb = {}
            for ci, (k0, k1) in enumerate(Y_CHUNKS):
                y_sb[ci] = pool.tile(
                    [P, k1 - k0, DIM], FP8, tag=f"y{ci}", name=f"y{ci}"
                )
            cb_sb = pool.tile([P, NT], F32, tag="cb")
            scr_sb = pool.tile([P, 2, 2 * P], FP8, tag="scr")
            # bf16 store staging: one [128, 2, 512] tile per feature block
            o_sb = [
                pool.tile([P, MCH, FREE], BF16, tag=f"o{n}", name=f"o{n}")
                for n in range(NT)
            ]

            agg_ps = [
                [
                    pspool.tile([P, FREE], F32, tag="ps", name=f"ps_{mc}_{n}")
                    for n in range(NT)
                ]
                for mc in range(MCH)
            ]

            # The whole input stream rides the ONE sync HWDGE queue in
            # exact consumption order: the SDMA packet round-robin between
            # queues is grossly unfair (big-transfer queues win ~10:1), so
            # a second queue for y starves the adj chunks the PE needs
            # next.  Single-queue FIFO makes arrival order deterministic
            # and one queue alone drains at full rate (~390 GB/s warm).
            nc.sync.dma_start(y_sb[0][:], yp_d[:, 0 : 2 * DIM])
            nc.sync.dma_start(adj0[0][:], adjp_d[:, 0 : TB // 2])
            nc.sync.dma_start(adj0[1][:], adjp_d[:, TB // 2 : TB])
            nc.sync.dma_start(cb_sb[:], cb_d[:])
            for ci, (t0, t1) in enumerate(ADJ_CHUNKS):
                k0, k1 = Y_CHUNKS[ci + 1]
                nc.sync.dma_start(y_sb[ci + 1][:], yp_d[:, k0 * DIM : k1 * DIM])
                nc.sync.dma_start(adj_sb[ci][:], adjp_d[:, t0 * TB : t1 * TB])

            nc.vector.memset(scr_sb[:], 0)

            # Dummy matmuls on zeroed scratch keep the PE busy from the
            # end of its preamble so the HAM clock gate reaches 8/8 before
            # real data lands.  They open bank (0,0)'s accumulation group
            # and add exact zeros, so no group boundary (and no scheduler
            # wait) sits between them and the real stream.
            for w in range(WARM):
                nc.tensor.matmul(
                    agg_ps[0][0][:, 0:P],
                    scr_sb[:, :, :P],
                    scr_sb[:, :, P:],
                    start=(w == 0),
                    stop=False,
                    perf_mode=DR,
                )

            def t_chunk(t):
                for ci, (t0, t1) in enumerate(ADJ_CHUNKS):
                    if t0 <= t < t1:
                        return ci, t0
                raise AssertionError

            def k_chunk(k):
                for ci, (k0, k1) in enumerate(Y_CHUNKS):
                    if k0 <= k < k1:
                        return ci, k0
                raise AssertionError

            for t in range(DT):
                yci, yk0 = k_chunk(2 * t)
                for n in range(NT):
                    y_ap = y_sb[yci][:, 2 * t - yk0 : 2 * t - yk0 + 2,
                                     n * P : (n + 1) * P]
                    for mc in range(MCH):
                        if t == 0:
                            a_ap = adj0[mc][:]
                        else:
                            aci, at0 = t_chunk(t)
                            a_ap = adj_sb[aci][:, t - at0, mc]
                        nc.tensor.matmul(
                            agg_ps[mc][n][:],
                            y_ap,
                            a_ap,
                            start=(t == 0) and not (n == 0 and mc == 0),
                            stop=(t == DT - 1),
                            perf_mode=DR,
                        )

            # Epilogue: bias+ReLU on PSUM eviction.  Bank (mc, n) stops at
            # the (n, mc)-ordered last t-tile MMs; DVE takes mc=0 (stops
            # first, slower op), ACT mc=1.  Each same-n pair stores as one
            # 256 KB HWDGE DMA the moment both halves land.  All stores on
            # sync (idle sequencer) except the last, which goes on scalar
            # AFTER its final RELU dispatch -- a store earlier in scalar's
            # FIFO would stall subsequent RELU dispatches, and gpsimd
            # (SWDGE) stores drag the end-of-kernel drain by ~2.5 us.
            store_eng = [nc.sync, nc.sync, nc.sync, nc.scalar]
            for n in range(NT):
                for mc in range(MCH):
                    dst = o_sb[n][:, mc]
                    if mc == 0:
                        nc.vector.tensor_scalar(
                            dst,
                            agg_ps[mc][n][:],
                            cb_sb[:, n : n + 1],
                            0.0,
                            mybir.AluOpType.add,
                            mybir.AluOpType.max,
                        )
                    else:
                        nc.scalar.activation(
                            dst,
                            agg_ps[mc][n][:],
                            mybir.ActivationFunctionType.Relu,
                            bias=cb_sb[:, n : n + 1],
                        )
                store_eng[n].dma_start(out_d[n], o_sb[n][:])
    nc.finalize()
    return nc


def _get_nc():
    global _NC
    if _NC is None:
        _NC = _build_nc()
    return _NC


def _prepare(inputs):
    e4 = ml_dtypes.float8_e4m3
    x = np.asarray(inputs["x"], dtype=np.float32)
    adj = np.asarray(inputs["adj"], dtype=np.float32)
    W = np.asarray(inputs["W"], dtype=np.float32)
    b = np.asarray(inputs["b"], dtype=np.float64)

    y = x @ W.T.astype(np.float32)
    y8 = y.astype(e4)
    # bias fold: nn bias + exact centering/rank-1 correction term
    c = (b + 0.5 * y.astype(np.float64).sum(axis=0)).astype(np.float32)
    cb_tiled = np.ascontiguousarray(c.reshape(NT, P).T)  # [128, 4]

    # y pre-tiled so every y DMA is flat: yp[p, k*DIM+d] = y8[k*128+p, d]
    yp = np.ascontiguousarray(
        y8.reshape(KT, P, DIM).transpose(1, 0, 2).reshape(P, KT * DIM)
    )

    B8T = (adj - np.float32(0.5)).astype(e4).T  # [K, rows] view

    in_maps = []
    for ci in range(NCORES):
        # adjp[p, t*2048 + mc*1024 + i*512 + m] = B8T[(2t+i)*128+p, ci*M+mc*512+m]
        shard = np.ascontiguousarray(B8T[:, ci * M : (ci + 1) * M])
        adjp = np.ascontiguousarray(
            shard.reshape(DT, 2, P, MCH, FREE).transpose(2, 0, 3, 1, 4)
        ).reshape(P, DT * TB)
        in_maps.append({"yp": yp, "adjp": adjp, "cb": cb_tiled})
    return in_maps


def _run(in_maps, **kwargs):
    return run_bass_kernel_spmd(
        _get_nc(), in_maps, core_ids=list(range(NCORES)), **kwargs
    )


def _assemble(results):
    out = np.empty((N_NODES, DIM), dtype=np.float32)
    for ci in range(NCORES):
        o5 = results[ci]["out5"].astype(np.float32)  # [NT, P, MCH*FREE]
        # element [n, p, j] = out[ci*1024 + j, n*128 + p]
        blk = o5.transpose(2, 0, 1).reshape(M, DIM)
        out[ci * M : (ci + 1) * M, :] = blk
    return out


def kernel(**inputs):
    res = _run(_prepare(inputs))
    return _assemble(res.results)


# revision 13
# speedup vs baseline: 1.0658x; 1.0127x over previous
"""Trainium2 Bass kernel: GNN message passing  out = relu((adj @ x) @ W.T + b).

Reassociated as  out = relu(adj @ y + b)  with y = x @ W.T folded into host
prep (0.8% of the FLOPs), so the device runs a single big matmul.  That
matmul runs in fp8e4 DoubleRow mode (2 weights per PE cell, 256-deep
contraction per instruction, ~216 ns per [256x128]x[256x512] MM = fp8 peak).
fp8 error is held at ~1.8e-2 by two exact algebraic corrections folded into
the bias:
  * adj is mean-centered (B = adj - 0.5), halving its quantization noise;
    the rank-1 term 0.5 * colsum(y) is exact.
  * using colsum(y_true) rather than colsum(y_fp8) also cancels the
    coherent (mean) component of y's quantization error, halving it.

Sharding: 1D row partition of adj across 8 NeuronCores (1024 rows each);
each core computes outT = y.T @ adjT_c with y-tiles stationary and the
centered adj shard streaming, accumulating f32 in all 8 PSUM banks over
the 8192-deep contraction, then fuses bias+ReLU on PSUM eviction.

v6 structure (from NTFF profile iteration; the PE body is at the fp8
roofline, so head/tail/DMA-pacing are what's left):
  * y and adj are HOST-INTERLEAVED into one flat per-partition-contiguous
    stream in exact consumption order (per t-tile: y[2t:2t+2] then the two
    adj mc-halves, 3 KB/partition) and ride the single sync HWDGE queue as
    ~0.4-1.5 MB chunk DMAs.  One queue alone drains at full rate, arrival
    order is deterministic (the SDMA packet round-robin between queues is
    grossly unfair), and big transfers keep the cold-window rate high --
    the PE consumes 384 KB per 1.73 us t-tile (~222 GB/s) and the stream
    must outrun that from the start.
  * the DMA ring does not move bytes before ~8.2 us regardless of issue
    time; the first chunk is exactly tile 0 (384 KB) so the first real MM
    fires ~10.7 us.  18 warm-up matmuls on zeroed scratch keep the PE
    busy from ~8.0 us so the HAM clock gate is at 8/8 when real data
    lands (cold real MMs cost ~1 us regardless -- busy can't start before
    the preamble ends, so the 3.4 us HAM window can't close earlier).
  * tail: the last t-tile's 8 MMs stop banks in (n, mc) order; DVE evicts
    mc=0 (stops first), ACT mc=1, pairs staged per n.  Pairs n0-n2 store
    as 256 KB DMAs on sync; the critical last pair is split into two
    128 KB halves on sync+scalar, each issued the moment its half lands.
    No gpsimd DMAs anywhere: SWDGE drags the end-of-kernel drain ~2.5 us.
Output is stored bf16 to halve the post-matmul store drain.
"""

import numpy as np
import ml_dtypes

import concourse.mybir as mybir
from concourse import bacc
from concourse.tile import TileContext
from concourse.bass_utils import run_bass_kernel_spmd

P = 128
N_NODES = 8192
DIM = 512
NCORES = 8
M = N_NODES // NCORES          # 1024 output rows per core
KT = N_NODES // P              # 64 contraction tiles of 128
DT = KT // 2                   # 32 DoubleRow tiles (256-deep each)
NT = DIM // P                  # 4 tiles of the feature dim (output part.)
FREE = 512                     # moving free dim / PSUM bank width (f32)
MCH = M // FREE                # 2 moving chunks per t-tile
CB = 3 * 2 * FREE              # stream bytes/partition per t (y + 2 adj)
WARM = 18                      # HAM warm-up matmuls on scratch
FP8 = mybir.dt.float8e4
F32 = mybir.dt.float32
BF16 = mybir.dt.bfloat16
DR = mybir.MatmulPerfMode.DoubleRow

# chunk boundaries in t units: first chunk = exactly tile 0, then growing
# sizes (0.4 -> 1.5 MB) balancing arrival granularity vs transfer rate.
CHUNKS = [(0, 1), (1, 2), (2, 4), (4, 6), (6, 8), (8, 10), (10, 12),
          (12, 16), (16, 20), (20, 24), (24, 28), (28, 32)]

_NC = None


def _build_nc():
    nc = bacc.Bacc("TRN2", debug=False)
    # big[p, t, 0, i, d]    = y[(2t+i)*128+p, d]
    # big[p, t, 1+mc, i, m] = B.T[(2t+i)*128+p, mc*512+m]
    big_d = nc.dram_tensor("big", [P, DT * CB], FP8, kind="ExternalInput").ap()
    cb_d = nc.dram_tensor("cb", [P, NT], F32, kind="ExternalInput").ap()
    # out5[n, p, mc*512+r] = outT[n*128+p, mc*512+r]
    out_d = nc.dram_tensor(
        "out5", [NT, P, MCH * FREE], BF16, kind="ExternalOutput"
    ).ap()

    with TileContext(nc) as tc:
        with (
            tc.tile_pool(name="sb", bufs=1) as pool,
            tc.tile_pool(name="ps", bufs=8, space="PSUM") as pspool,
        ):
            str_sb = {}
            for ci, (t0, t1) in enumerate(CHUNKS):
                str_sb[ci] = pool.tile(
                    [P, t1 - t0, 3, 2, FREE], FP8, tag=f"str{ci}", name=f"str{ci}"
                )
            cb_sb = pool.tile([P, NT], F32, tag="cb")
            scr_sb = pool.tile([P, 2, 2 * P], FP8, tag="scr")
            o_sb = [
                pool.tile([P, MCH, FREE], BF16, tag=f"o{n}", name=f"o{n}")
                for n in range(NT)
            ]

            agg_ps = [
                [
                    pspool.tile([P, FREE], F32, tag="ps", name=f"ps_{mc}_{n}")
                    for n in range(NT)
                ]
                for mc in range(MCH)
            ]

            # The whole input stream rides the ONE sync HWDGE queue in
            # exact consumption order; bias rides behind it (needed only
            # by the epilogue).
            for ci, (t0, t1) in enumerate(CHUNKS):
                nc.sync.dma_start(str_sb[ci][:], big_d[:, t0 * CB : t1 * CB])
            nc.sync.dma_start(cb_sb[:], cb_d[:])

            nc.vector.memset(scr_sb[:], 0)

            # Dummy matmuls on zeroed scratch keep the PE busy from the
            # end of its preamble so the HAM clock gate reaches 8/8 before
            # real data lands.  They open bank (0,0)'s accumulation group
            # and add exact zeros, so no group boundary (and no scheduler
            # wait) sits between them and the real stream.
            for w in range(WARM):
                nc.tensor.matmul(
                    agg_ps[0][0][:, 0:P],
                    scr_sb[:, :, :P],
                    scr_sb[:, :, P:],
                    start=(w == 0),
                    stop=False,
                    perf_mode=DR,
                )

            def t_chunk(t):
                for ci, (t0, t1) in enumerate(CHUNKS):
                    if t0 <= t < t1:
                        return ci, t0
                raise AssertionError

            for t in range(DT):
                ci, t0 = t_chunk(t)
                tile = str_sb[ci]
                j = t - t0
                for n in range(NT):
                    y_ap = tile[:, j, 0, :, n * P : (n + 1) * P]
                    for mc in range(MCH):
                        nc.tensor.matmul(
                            agg_ps[mc][n][:],
                            y_ap,
                            tile[:, j, 1 + mc],
                            start=(t == 0) and not (n == 0 and mc == 0),
                            stop=(t == DT - 1),
                            perf_mode=DR,
                        )

            # Epilogue: bias+ReLU on PSUM eviction.  Bank (mc, n) stops at
            # the (n, mc)-ordered last t-tile MMs; DVE takes mc=0 (stops
            # first), ACT mc=1.  Pairs n0-n2 store as one 256 KB HWDGE DMA
            # on sync (idle sequencer); the critical last pair splits into
            # two 128 KB halves on sync+scalar, each gated on only its own
            # eviction.  A store earlier in scalar's FIFO would stall
            # subsequent RELU dispatches; gpsimd (SWDGE) stores drag the
            # end-of-kernel drain by ~2.5 us.
            for n in range(NT):
                for mc in range(MCH):
                    dst = o_sb[n][:, mc]
                    if mc == 0:
                        nc.vector.tensor_scalar(
                            dst,
                            agg_ps[mc][n][:],
                            cb_sb[:, n : n + 1],
                            0.0,
                            mybir.AluOpType.add,
                            mybir.AluOpType.max,
                        )
                    else:
                        nc.scalar.activation(
                            dst,
                            agg_ps[mc][n][:],
                            mybir.ActivationFunctionType.Relu,
                            bias=cb_sb[:, n : n + 1],
                        )
                if n < NT - 1:
                    nc.sync.dma_start(out_d[n], o_sb[n][:])
                else:
                    nc.sync.dma_start(out_d[n][:, 0:FREE], o_sb[n][:, 0])
                    nc.scalar.dma_start(out_d[n][:, FREE : 2 * FREE], o_sb[n][:, 1])
    nc.finalize()
    return nc


def _get_nc():
    global _NC
    if _NC is None:
        _NC = _build_nc()
    return _NC


def _prepare(inputs):
    e4 = ml_dtypes.float8_e4m3
    x = np.asarray(inputs["x"], dtype=np.float32)
    adj = np.asarray(inputs["adj"], dtype=np.float32)
    W = np.asarray(inputs["W"], dtype=np.float32)
    b = np.asarray(inputs["b"], dtype=np.float64)

    y = x @ W.T.astype(np.float32)
    y8 = y.astype(e4)
    # bias fold: nn bias + exact centering/rank-1 correction term
    c = (b + 0.5 * y.astype(np.float64).sum(axis=0)).astype(np.float32)
    cb_tiled = np.ascontiguousarray(c.reshape(NT, P).T)  # [128, 4]

    # y part of the stream: [p, t, i, d] = y8[(2t+i)*128+p, d]
    ypart = y8.reshape(DT, 2, P, DIM).transpose(2, 0, 1, 3)[:, :, None]

    B8T = (adj - np.float32(0.5)).astype(e4).T  # [K, rows] view

    in_maps = []
    for ci in range(NCORES):
        shard = np.ascontiguousarray(B8T[:, ci * M : (ci + 1) * M])
        # adj part: [p, t, mc, i, m] = B8T[(2t+i)*128+p, ci*M + mc*512+m]
        apart = shard.reshape(DT, 2, P, MCH, FREE).transpose(2, 0, 3, 1, 4)
        big = np.ascontiguousarray(
            np.concatenate([ypart, apart], axis=2)
        ).reshape(P, DT * CB)
        in_maps.append({"big": big, "cb": cb_tiled})
    return in_maps


def _run(in_maps, **kwargs):
    return run_bass_kernel_spmd(
        _get_nc(), in_maps, core_ids=list(range(NCORES)), **kwargs
    )


def _assemble(results):
    out = np.empty((N_NODES, DIM), dtype=np.float32)
    for ci in range(NCORES):
        o5 = results[ci]["out5"].astype(np.float32)  # [NT, P, MCH*FREE]
        # element [n, p, j] = out[ci*1024 + j, n*128 + p]
        blk = o5.transpose(2, 0, 1).reshape(M, DIM)
        out[ci * M : (ci + 1) * M, :] = blk
    return out


def kernel(**inputs):
    res = _run(_prepare(inputs))
    return _assemble(res.results)


# revision 15
# speedup vs baseline: 1.0715x; 1.0053x over previous
"""Trainium2 Bass kernel: GNN message passing  out = relu((adj @ x) @ W.T + b).

Reassociated as  out = relu(adj @ y + b)  with y = x @ W.T folded into host
prep (0.8% of the FLOPs), so the device runs a single big matmul.  That
matmul runs in fp8e4 DoubleRow mode (2 weights per PE cell, 256-deep
contraction per instruction, ~216 ns per [256x128]x[256x512] MM = fp8 peak).
fp8 error is held at ~1.8e-2 by two exact algebraic corrections folded into
the bias:
  * adj is mean-centered (B = adj - 0.5), halving its quantization noise;
    the rank-1 term 0.5 * colsum(y) is exact.
  * using colsum(y_true) rather than colsum(y_fp8) also cancels the
    coherent (mean) component of y's quantization error, halving it.

Sharding: 1D row partition of adj across 8 NeuronCores (1024 rows each);
each core computes outT = y.T @ adjT_c with y-tiles stationary and the
centered adj shard streaming, accumulating f32 in all 8 PSUM banks over
the 8192-deep contraction, then fuses bias+ReLU on PSUM eviction.

v6 structure (from NTFF profile iteration; the PE body is at the fp8
roofline, so head/tail/DMA-pacing are what's left):
  * y and adj are HOST-INTERLEAVED into one flat per-partition-contiguous
    stream in exact consumption order (per t-tile: y[2t:2t+2] then the two
    adj mc-halves, 3 KB/partition) and ride the single sync HWDGE queue as
    ~0.4-1.5 MB chunk DMAs.  One queue alone drains at full rate, arrival
    order is deterministic (the SDMA packet round-robin between queues is
    grossly unfair), and big transfers keep the cold-window rate high --
    the PE consumes 384 KB per 1.73 us t-tile (~222 GB/s) and the stream
    must outrun that from the start.
  * the DMA ring does not move bytes before ~8.2 us regardless of issue
    time; the first chunk is exactly tile 0 (384 KB) so the first real MM
    fires ~10.7 us.  18 warm-up matmuls on zeroed scratch keep the PE
    busy from ~8.0 us so the HAM clock gate is at 8/8 when real data
    lands (cold real MMs cost ~1 us regardless -- busy can't start before
    the preamble ends, so the 3.4 us HAM window can't close earlier).
  * tail: the last t-tile's 8 MMs stop banks in (n, mc) order; DVE evicts
    mc=0 (stops first), ACT mc=1, pairs staged per n.  Pairs n0-n2 store
    as 256 KB DMAs on sync; the critical last pair is split into two
    128 KB halves on sync+scalar, each issued the moment its half lands.
    No gpsimd DMAs anywhere: SWDGE drags the end-of-kernel drain ~2.5 us.
Output is stored bf16 to halve the post-matmul store drain.
"""

import numpy as np
import ml_dtypes

import concourse.mybir as mybir
from concourse import bacc
from concourse.tile import TileContext
from concourse.bass_utils import run_bass_kernel_spmd

P = 128
N_NODES = 8192
DIM = 512
NCORES = 8
M = N_NODES // NCORES          # 1024 output rows per core
KT = N_NODES // P              # 64 contraction tiles of 128
DT = KT // 2                   # 32 DoubleRow tiles (256-deep each)
NT = DIM // P                  # 4 tiles of the feature dim (output part.)
FREE = 512                     # moving free dim / PSUM bank width (f32)
MCH = M // FREE                # 2 moving chunks per t-tile
CB = 3 * 2 * FREE              # stream bytes/partition per t (y + 2 adj)
WARM = 24                      # HAM warm-up matmuls on scratch
FP8 = mybir.dt.float8e4
F32 = mybir.dt.float32
BF16 = mybir.dt.bfloat16
DR = mybir.MatmulPerfMode.DoubleRow

# chunk boundaries in t units: first chunk = exactly tile 0, then growing
# sizes (0.4 -> 1.5 MB) balancing arrival granularity vs transfer rate.
CHUNKS = [(0, 1), (1, 2), (2, 4), (4, 6), (6, 8), (8, 10), (10, 12),
          (12, 16), (16, 20), (20, 24), (24, 28), (28, 32)]

_NC = None


def _build_nc():
    nc = bacc.Bacc("TRN2", debug=False)
    # big[p, t, 0, i, d]    = y[(2t+i)*128+p, d]
    # big[p, t, 1+mc, i, m] = B.T[(2t+i)*128+p, mc*512+m]
    big_d = nc.dram_tensor("big", [P, DT * CB], FP8, kind="ExternalInput").ap()
    cb_d = nc.dram_tensor("cb", [P, NT], F32, kind="ExternalInput").ap()
    # out5[n, p, mc*512+r] = outT[n*128+p, mc*512+r]
    out_d = nc.dram_tensor(
        "out5", [NT, P, MCH * FREE], BF16, kind="ExternalOutput"
    ).ap()

    with TileContext(nc) as tc:
        with (
            tc.tile_pool(name="sb", bufs=1) as pool,
            tc.tile_pool(name="ps", bufs=8, space="PSUM") as pspool,
        ):
            str_sb = {}
            for ci, (t0, t1) in enumerate(CHUNKS):
                str_sb[ci] = pool.tile(
                    [P, t1 - t0, 3, 2, FREE], FP8, tag=f"str{ci}", name=f"str{ci}"
                )
            cb_sb = pool.tile([P, NT], F32, tag="cb")
            scr_sb = pool.tile([P, 2, 2 * P], FP8, tag="scr")
            o_sb = [
                pool.tile([P, MCH, FREE], BF16, tag=f"o{n}", name=f"o{n}")
                for n in range(NT)
            ]

            agg_ps = [
                [
                    pspool.tile([P, FREE], F32, tag="ps", name=f"ps_{mc}_{n}")
                    for n in range(NT)
                ]
                for mc in range(MCH)
            ]

            # The whole input stream rides the ONE sync HWDGE queue in
            # exact consumption order; bias rides behind it (needed only
            # by the epilogue).
            # chunk 0 split: the first MM is gated on y(t0)+adj(t0,mc0)
            # only (256 KB); adj(t0,mc1) follows as its own transfer.
            nc.sync.dma_start(str_sb[0][:, 0, 0:2], big_d[:, 0 : 2 * 2 * FREE])
            nc.sync.dma_start(str_sb[0][:, 0, 2], big_d[:, 2 * 2 * FREE : CB])
            for ci, (t0, t1) in enumerate(CHUNKS):
                if ci:
                    nc.sync.dma_start(str_sb[ci][:], big_d[:, t0 * CB : t1 * CB])
            nc.sync.dma_start(cb_sb[:], cb_d[:])

            nc.vector.memset(scr_sb[:], 0)

            # Dummy matmuls on zeroed scratch keep the PE busy from the
            # end of its preamble so the HAM clock gate reaches 8/8 before
            # real data lands.  They open bank (0,0)'s accumulation group
            # and add exact zeros, so no group boundary (and no scheduler
            # wait) sits between them and the real stream.
            for w in range(WARM):
                nc.tensor.matmul(
                    agg_ps[0][0][:, 0:P],
                    scr_sb[:, :, :P],
                    scr_sb[:, :, P:],
                    start=(w == 0),
                    stop=False,
                    perf_mode=DR,
                )

            def t_chunk(t):
                for ci, (t0, t1) in enumerate(CHUNKS):
                    if t0 <= t < t1:
                        return ci, t0
                raise AssertionError

            for t in range(DT):
                ci, t0 = t_chunk(t)
                tile = str_sb[ci]
                j = t - t0
                for n in range(NT):
                    y_ap = tile[:, j, 0, :, n * P : (n + 1) * P]
                    for mc in range(MCH):
                        nc.tensor.matmul(
                            agg_ps[mc][n][:],
                            y_ap,
                            tile[:, j, 1 + mc],
                            start=(t == 0) and not (n == 0 and mc == 0),
                            stop=(t == DT - 1),
                            perf_mode=DR,
                        )

            # Epilogue: bias+ReLU on PSUM eviction.  Bank (mc, n) stops at
            # the (n, mc)-ordered last t-tile MMs; DVE takes mc=0 (stops
            # first), ACT mc=1.  Pairs n0-n2 store as one 256 KB HWDGE DMA
            # on sync (idle sequencer); the critical last pair splits into
            # two 128 KB halves on sync+scalar, each gated on only its own
            # eviction.  A store earlier in scalar's FIFO would stall
            # subsequent RELU dispatches; gpsimd (SWDGE) stores drag the
            # end-of-kernel drain by ~2.5 us.
            for n in range(NT):
                for mc in range(MCH):
                    dst = o_sb[n][:, mc]
                    if mc == 0:
                        nc.vector.tensor_scalar(
                            dst,
                            agg_ps[mc][n][:],
                            cb_sb[:, n : n + 1],
                            0.0,
                            mybir.AluOpType.add,
                            mybir.AluOpType.max,
                        )
                    else:
                        nc.scalar.activation(
                            dst,
                            agg_ps[mc][n][:],
                            mybir.ActivationFunctionType.Relu,
                            bias=cb_sb[:, n : n + 1],
                        )
                if n < NT - 1:
                    nc.sync.dma_start(out_d[n], o_sb[n][:])
                else:
                    nc.sync.dma_start(out_d[n][:, 0:FREE], o_sb[n][:, 0])
                    nc.scalar.dma_start(out_d[n][:, FREE : 2 * FREE], o_sb[n][:, 1])
    nc.finalize()
    return nc


def _get_nc():
    global _NC
    if _NC is None:
        _NC = _build_nc()
    return _NC


def _prepare(inputs):
    e4 = ml_dtypes.float8_e4m3
    x = np.asarray(inputs["x"], dtype=np.float32)
    adj = np.asarray(inputs["adj"], dtype=np.float32)
    W = np.asarray(inputs["W"], dtype=np.float32)
    b = np.asarray(inputs["b"], dtype=np.float64)

    y = x @ W.T.astype(np.float32)
    y8 = y.astype(e4)
    # bias fold: nn bias + exact centering/rank-1 correction term
    c = (b + 0.5 * y.astype(np.float64).sum(axis=0)).astype(np.float32)
    cb_tiled = np.ascontiguousarray(c.reshape(NT, P).T)  # [128, 4]

    # y part of the stream: [p, t, i, d] = y8[(2t+i)*128+p, d]
    ypart = y8.reshape(DT, 2, P, DIM).transpose(2, 0, 1, 3)[:, :, None]

    B8T = (adj - np.float32(0.5)).astype(e4).T  # [K, rows] view

    in_maps = []
    for ci in range(NCORES):
        shard = np.ascontiguousarray(B8T[:, ci * M : (ci + 1) * M])
        # adj part: [p, t, mc, i, m] = B8T[(2t+i)*128+p, ci*M + mc*512+m]
        apart = shard.reshape(DT, 2, P, MCH, FREE).transpose(2, 0, 3, 1, 4)
        big = np.ascontiguousarray(
            np.concatenate([ypart, apart], axis=2)
        ).reshape(P, DT * CB)
        in_maps.append({"big": big, "cb": cb_tiled})
    return in_maps


def _run(in_maps, **kwargs):
    return run_bass_kernel_spmd(
        _get_nc(), in_maps, core_ids=list(range(NCORES)), **kwargs
    )


def _assemble(results):
    out = np.empty((N_NODES, DIM), dtype=np.float32)
    for ci in range(NCORES):
        o5 = results[ci]["out5"].astype(np.float32)  # [NT, P, MCH*FREE]
        # element [n, p, j] = out[ci*1024 + j, n*128 + p]
        blk = o5.transpose(2, 0, 1).reshape(M, DIM)
        out[ci * M : (ci + 1) * M, :] = blk
    return out


def kernel(**inputs):
    res = _run(_prepare(inputs))
    return _assemble(res.results)


# revision 17
# speedup vs baseline: 1.0770x; 1.0051x over previous
"""Trainium2 Bass kernel: GNN message passing  out = relu((adj @ x) @ W.T + b).

Reassociated as  out = relu(adj @ y + b)  with y = x @ W.T folded into host
prep (0.8% of the FLOPs), so the device runs a single big matmul.  That
matmul runs in fp8e4 DoubleRow mode (2 weights per PE cell, 256-deep
contraction per instruction, ~216 ns per [256x128]x[256x512] MM = fp8 peak).
fp8 error is held at ~1.8e-2 by two exact algebraic corrections folded into
the bias:
  * adj is mean-centered (B = adj - 0.5), halving its quantization noise;
    the rank-1 term 0.5 * colsum(y) is exact.
  * using colsum(y_true) rather than colsum(y_fp8) also cancels the
    coherent (mean) component of y's quantization error, halving it.

Sharding: 1D row partition of adj across 8 NeuronCores (1024 rows each);
each core computes outT = y.T @ adjT_c with y-tiles stationary and the
centered adj shard streaming, accumulating f32 in all 8 PSUM banks over
the 8192-deep contraction, then fuses bias+ReLU on PSUM eviction.

v6 structure (from NTFF profile iteration; the PE body is at the fp8
roofline, so head/tail/DMA-pacing are what's left):
  * y and adj are HOST-INTERLEAVED into one flat per-partition-contiguous
    stream in exact consumption order (per t-tile: y[2t:2t+2] then the two
    adj mc-halves, 3 KB/partition) and ride the single sync HWDGE queue as
    ~0.4-1.5 MB chunk DMAs.  One queue alone drains at full rate, arrival
    order is deterministic (the SDMA packet round-robin between queues is
    grossly unfair), and big transfers keep the cold-window rate high --
    the PE consumes 384 KB per 1.73 us t-tile (~222 GB/s) and the stream
    must outrun that from the start.
  * the DMA ring does not move bytes before ~8.2 us regardless of issue
    time; the first chunk is exactly tile 0 (384 KB) so the first real MM
    fires ~10.7 us.  18 warm-up matmuls on zeroed scratch keep the PE
    busy from ~8.0 us so the HAM clock gate is at 8/8 when real data
    lands (cold real MMs cost ~1 us regardless -- busy can't start before
    the preamble ends, so the 3.4 us HAM window can't close earlier).
  * tail: the last t-tile's 8 MMs stop banks in (n, mc) order; DVE evicts
    mc=0 (stops first), ACT mc=1, pairs staged per n.  Pairs n0-n2 store
    as 256 KB DMAs on sync; the critical last pair is split into two
    128 KB halves on sync+scalar, each issued the moment its half lands.
    No gpsimd DMAs anywhere: SWDGE drags the end-of-kernel drain ~2.5 us.
Output is stored bf16 to halve the post-matmul store drain.
"""

import numpy as np
import ml_dtypes

import concourse.mybir as mybir
from concourse import bacc
from concourse.tile import TileContext
from concourse.bass_utils import run_bass_kernel_spmd

P = 128
N_NODES = 8192
DIM = 512
NCORES = 8
M = N_NODES // NCORES          # 1024 output rows per core
KT = N_NODES // P              # 64 contraction tiles of 128
DT = KT // 2                   # 32 DoubleRow tiles (256-deep each)
NT = DIM // P                  # 4 tiles of the feature dim (output part.)
FREE = 512                     # moving free dim / PSUM bank width (f32)
MCH = M // FREE                # 2 moving chunks per t-tile
CB = 3 * 2 * FREE              # stream bytes/partition per t (y + 2 adj)
WARM = 28                      # HAM warm-up matmuls on scratch
FP8 = mybir.dt.float8e4
F32 = mybir.dt.float32
BF16 = mybir.dt.bfloat16
DR = mybir.MatmulPerfMode.DoubleRow

# chunk boundaries in t units: first chunk = exactly tile 0, then growing
# sizes (0.4 -> 1.5 MB) balancing arrival granularity vs transfer rate.
CHUNKS = [(0, 1), (1, 2), (2, 4), (4, 6), (6, 8), (8, 10), (10, 12),
          (12, 16), (16, 20), (20, 24), (24, 28), (28, 32)]

_NC = None


def _build_nc():
    nc = bacc.Bacc("TRN2", debug=False)
    # big[p, t, 0, i, d]    = y[(2t+i)*128+p, d]
    # big[p, t, 1+mc, i, m] = B.T[(2t+i)*128+p, mc*512+m]
    big_d = nc.dram_tensor("big", [P, DT * CB], FP8, kind="ExternalInput").ap()
    cb_d = nc.dram_tensor("cb", [P, NT], F32, kind="ExternalInput").ap()
    # out5[n, p, mc*512+r] = outT[n*128+p, mc*512+r]
    out_d = nc.dram_tensor(
        "out5", [NT, P, MCH * FREE], BF16, kind="ExternalOutput"
    ).ap()

    with TileContext(nc) as tc:
        with (
            tc.tile_pool(name="sb", bufs=1) as pool,
            tc.tile_pool(name="ps", bufs=8, space="PSUM") as pspool,
        ):
            str_sb = {}
            for ci, (t0, t1) in enumerate(CHUNKS):
                str_sb[ci] = pool.tile(
                    [P, t1 - t0, 3, 2, FREE], FP8, tag=f"str{ci}", name=f"str{ci}"
                )
            cb_sb = pool.tile([P, NT], F32, tag="cb")
            scr_sb = pool.tile([P, 2, 2 * P], FP8, tag="scr")
            o_sb = [
                pool.tile([P, MCH, FREE], BF16, tag=f"o{n}", name=f"o{n}")
                for n in range(NT)
            ]

            agg_ps = [
                [
                    pspool.tile([P, FREE], F32, tag="ps", name=f"ps_{mc}_{n}")
                    for n in range(NT)
                ]
                for mc in range(MCH)
            ]

            # The whole input stream rides the ONE sync HWDGE queue in
            # exact consumption order; bias rides behind it (needed only
            # by the epilogue).
            for ci, (t0, t1) in enumerate(CHUNKS):
                nc.sync.dma_start(str_sb[ci][:], big_d[:, t0 * CB : t1 * CB])
            nc.sync.dma_start(cb_sb[:], cb_d[:])

            # memset on gpsimd: it is otherwise idle and starts right at
            # main, so the PE's warm-up chain (and with it the 3.4 us HAM
            # busy window) starts ~0.5 us earlier than a DVE memset allows.
            nc.gpsimd.memset(scr_sb[:], 0)

            # Dummy matmuls on zeroed scratch keep the PE busy from the
            # end of its preamble so the HAM clock gate reaches 8/8 before
            # real data lands.  They open bank (0,0)'s accumulation group
            # and add exact zeros, so no group boundary (and no scheduler
            # wait) sits between them and the real stream.
            for w in range(WARM):
                nc.tensor.matmul(
                    agg_ps[0][0][:, 0:P],
                    scr_sb[:, :, :P],
                    scr_sb[:, :, P:],
                    start=(w == 0),
                    stop=False,
                    perf_mode=DR,
                )

            def t_chunk(t):
                for ci, (t0, t1) in enumerate(CHUNKS):
                    if t0 <= t < t1:
                        return ci, t0
                raise AssertionError

            for t in range(DT):
                ci, t0 = t_chunk(t)
                tile = str_sb[ci]
                j = t - t0
                for n in range(NT):
                    y_ap = tile[:, j, 0, :, n * P : (n + 1) * P]
                    for mc in range(MCH):
                        nc.tensor.matmul(
                            agg_ps[mc][n][:],
                            y_ap,
                            tile[:, j, 1 + mc],
                            start=(t == 0) and not (n == 0 and mc == 0),
                            stop=(t == DT - 1),
                            perf_mode=DR,
                        )

            # Epilogue: bias+ReLU on PSUM eviction.  Bank (mc, n) stops at
            # the (n, mc)-ordered last t-tile MMs; DVE takes mc=0 (stops
            # first), ACT mc=1.  Pairs n0-n2 store as one 256 KB HWDGE DMA
            # on sync (idle sequencer); the critical last pair splits into
            # two 128 KB halves on sync+scalar, each gated on only its own
            # eviction.  A store earlier in scalar's FIFO would stall
            # subsequent RELU dispatches; gpsimd (SWDGE) stores drag the
            # end-of-kernel drain by ~2.5 us.
            for n in range(NT):
                for mc in range(MCH):
                    dst = o_sb[n][:, mc]
                    if mc == 0:
                        nc.vector.tensor_scalar(
                            dst,
                            agg_ps[mc][n][:],
                            cb_sb[:, n : n + 1],
                            0.0,
                            mybir.AluOpType.add,
                            mybir.AluOpType.max,
                        )
                    else:
                        nc.scalar.activation(
                            dst,
                            agg_ps[mc][n][:],
                            mybir.ActivationFunctionType.Relu,
                            bias=cb_sb[:, n : n + 1],
                        )
                if n < NT - 1:
                    nc.sync.dma_start(out_d[n], o_sb[n][:])
                else:
                    nc.sync.dma_start(out_d[n][:, 0:FREE], o_sb[n][:, 0])
                    nc.scalar.dma_start(out_d[n][:, FREE : 2 * FREE], o_sb[n][:, 1])
    nc.finalize()
    return nc


def _get_nc():
    global _NC
    if _NC is None:
        _NC = _build_nc()
    return _NC


def _prepare(inputs):
    e4 = ml_dtypes.float8_e4m3
    x = np.asarray(inputs["x"], dtype=np.float32)
    adj = np.asarray(inputs["adj"], dtype=np.float32)
    W = np.asarray(inputs["W"], dtype=np.float32)
    b = np.asarray(inputs["b"], dtype=np.float64)

    y = x @ W.T.astype(np.float32)
    y8 = y.astype(e4)
    # bias fold: nn bias + exact centering/rank-1 correction term
    c = (b + 0.5 * y.astype(np.float64).sum(axis=0)).astype(np.float32)
    cb_tiled = np.ascontiguousarray(c.reshape(NT, P).T)  # [128, 4]

    # y part of the stream: [p, t, i, d] = y8[(2t+i)*128+p, d]
    ypart = y8.reshape(DT, 2, P, DIM).transpose(2, 0, 1, 3)[:, :, None]

    B8T = (adj - np.float32(0.5)).astype(e4).T  # [K, rows] view

    in_maps = []
    for ci in range(NCORES):
        shard = np.ascontiguousarray(B8T[:, ci * M : (ci + 1) * M])
        # adj part: [p, t, mc, i, m] = B8T[(2t+i)*128+p, ci*M + mc*512+m]
        apart = shard.reshape(DT, 2, P, MCH, FREE).transpose(2, 0, 3, 1, 4)
        big = np.ascontiguousarray(
            np.concatenate([ypart, apart], axis=2)
        ).reshape(P, DT * CB)
        in_maps.append({"big": big, "cb": cb_tiled})
    return in_maps


def _run(in_maps, **kwargs):
    return run_bass_kernel_spmd(
        _get_nc(), in_maps, core_ids=list(range(NCORES)), **kwargs
    )


def _assemble(results):
    out = np.empty((N_NODES, DIM), dtype=np.float32)
    for ci in range(NCORES):
        o5 = results[ci]["out5"].astype(np.float32)  # [NT, P, MCH*FREE]
        # element [n, p, j] = out[ci*1024 + j, n*128 + p]
        blk = o5.transpose(2, 0, 1).reshape(M, DIM)
        out[ci * M : (ci + 1) * M, :] = blk
    return out


def kernel(**inputs):
    res = _run(_prepare(inputs))
    return _assemble(res.results)
